# revision 30
# baseline (speedup 1.0000x reference)
"""Distributed Trainium2 kernel for nn_Attention_25228637897408.

GQA attention (B=1, T=2048, D=2048, NH=16, NKV=4, HD=128) with RoPE,
per-head rms_norm, skip-gate blend of k/v, v_bias, causal softmax and
output projection, tensor-parallel over heads on 8 NeuronCores.

Per-core work (core c): q-heads {2c, 2c+1}, kv-head c//2.

Key structure (v2, rewritten for PE efficiency):
  - skip-gate blend folded into the INPUT: xb' = x + (g/(1-g))*skip with
    (1-g) folded into wk/wv, so k = xb'@((1-g)wk).T and v likewise each
    take one pass over D (the baseline did two).  q keeps raw x/wq.  The
    blend is 2 in-place DVE passes over the skip tiles, interleaved with
    the unit loop so the DVE FIFO never blocks the norm epilogues.
  - partition-dim reductions (rms-norm sum-of-squares, softmax denom l)
    are ones-stationary matmuls into [1,512] PSUM rows - one 512-wide MM
    instead of 4x 1-col MMs + LDWEIGHTS storms.  The row is then
    partition-broadcast with a rank-1 matmul and inverted with a
    full-lane DVE reciprocal.
  - causal mask applied as a DVE multiply by a 0/1 mask AFTER exp
    (safe: exp(gsc*s - C) <= 1 for all entries since the diagonal
    attains the bound), removing the PE mask-add matmuls.
  - phase 2 runs a depth-2 software pipeline (scores pair i on PE while
    exp of pair i-1 runs on ACT and l/yT of pair i-2 on PE) so the
    scalar engine's exp fully hides under matmuls.
  - all x/skip DMAs are issued up front into independent buffers
    (no half-1 starvation window); wproj streams on the gpsimd queue
    during phase 2.
  - v is transposed to natural layout with the XBAR dma transpose, not
    the PE.
  - phase 3 accumulates head-0 din-blocks while head-1's AllToAll is in
    flight (8 PSUM banks hold the full output tile).
Host side only reshapes/transposes/casts and slices inputs; all value
computation (sigmoid, blending, norms, softmax, matmuls) is on device.
"""

import sys

sys.path.insert(0, "/opt/trn_rl_repo")

from collections import deque

import numpy as np
import ml_dtypes

import concourse.bass as bass
import concourse.mybir as mybir
import concourse.tile as tile
from concourse import bacc
from concourse.bass_utils import run_bass_kernel_spmd

BF16 = ml_dtypes.bfloat16

T = 2048
D = 2048
NH = 16
NKV = 4
HD = 128
REP = NH // NKV
NCORES = 8
HQ = NH // NCORES  # q heads per core = 2
ROPE_BASE = 10000.0
EPS = float(np.finfo(np.float32).eps)
N_FILL = 80

dt = mybir.dt
AF = mybir.ActivationFunctionType
ALU = mybir.AluOpType


def _bf(x):
    return np.ascontiguousarray(np.asarray(x, dtype=np.float32)).astype(BF16)


def build_graph(t=T):
    assert t % 1024 == 0
    n_chunk = t // 512  # 512-wide q chunks
    n_kt = t // 128  # 128-row tiles along T
    kpc = n_kt // n_chunk  # k-tiles per chunk = 4
    rows = t // NCORES  # output rows per core
    n_dt = D // 128  # tiles along D contraction = 16
    n_half = t // 1024

    nc = bacc.Bacc(None, target_bir_lowering=False)

    xT_d = nc.declare_dram_parameter("xT", [D, t], dt.bfloat16, isOutput=False)
    skT_d = nc.declare_dram_parameter("skipT", [D, t], dt.bfloat16, isOutput=False)
    wqT_d = nc.declare_dram_parameter("wqT", [D, HQ * HD], dt.bfloat16, isOutput=False)
    wkT_d = nc.declare_dram_parameter("wkT", [D, HD], dt.bfloat16, isOutput=False)
    wvT_d = nc.declare_dram_parameter("wvT", [D, HD], dt.bfloat16, isOutput=False)
    wpT_d = nc.declare_dram_parameter("wprojT", [D, D], dt.bfloat16, isOutput=False)
    qkg_d = nc.declare_dram_parameter("qkg", [1, HQ], dt.float32, isOutput=False)
    lns_d = nc.declare_dram_parameter("lns", [1, 1], dt.float32, isOutput=False)
    vb_d = nc.declare_dram_parameter("vbias", [1, HD], dt.float32, isOutput=False)
    cosF_d = nc.declare_dram_parameter("cosF", [HD, t], dt.bfloat16, isOutput=False)
    sinF_d = nc.declare_dram_parameter("sinF", [HD, t], dt.bfloat16, isOutput=False)
    mask_d = nc.declare_dram_parameter("maskb", [128, kpc * 512], dt.bfloat16, isOutput=False)
    id_d = nc.declare_dram_parameter("ident", [128, 128], dt.bfloat16, isOutput=False)
    out_d = nc.declare_dram_parameter("out", [rows, D], dt.float32, isOutput=True)

    with tile.TileContext(nc) as tc:
        with (
            tc.tile_pool(name="consts", bufs=1) as cp,
            tc.tile_pool(name="dram", bufs=1, space="DRAM") as dp,
        ):
            # ---- early constants (fillers depend only on these) ----
            ident = cp.tile([128, 128], dt.bfloat16, tag="ident")
            mask = cp.tile([128, kpc * 512], dt.bfloat16, tag="mask")
            nc.sync.dma_start(out=ident[:], in_=id_d[:])
            nc.sync.dma_start(out=mask[:], in_=mask_d[:])

            qkg = cp.tile([1, HQ], dt.float32, tag="qkg")
            lns = cp.tile([1, 1], dt.float32, tag="lns")
            vb = cp.tile([1, HD], dt.float32, tag="vb")
            nc.sync.dma_start(out=qkg[:], in_=qkg_d[:])
            nc.sync.dma_start(out=lns[:], in_=lns_d[:])
            nc.sync.dma_start(out=vb[:], in_=vb_d[:])

            # phase-1 PSUM pools (8 banks: proj 3 + ssq 2 + rb 2 + spare 1)
            psp = tc.alloc_tile_pool(name="p1ps", bufs=3, space="PSUM")
            ssqp = tc.alloc_tile_pool(name="ssq_ps", bufs=2, space="PSUM")
            rbp = tc.alloc_tile_pool(name="rb_ps", bufs=2, space="PSUM")

            # fillers: dependency-chained dummy matmuls keep the PE HAM-warm
            # through the initial DMA window. They use the rb pool's first
            # buffer (free until the first epilogue completes).  The 0-scaled
            # read into `junk` keeps them reachable from a real output so the
            # compiler cannot dead-code them away.
            fps = rbp.tile([128, 512], dt.float32, tag="rb")
            for i in range(N_FILL):
                nc.tensor.matmul(fps[:], lhsT=ident[:], rhs=mask[:, 0:512], start=(i == 0), stop=(i == N_FILL - 1))
            junk = cp.tile([1, 1], dt.float32, tag="junk")
            nc.vector.tensor_scalar_mul(junk[:], fps[0:1, 0:1], 0.0)

            # ---- small scalars ----
            ones_col = cp.tile([128, 1], dt.bfloat16, tag="ones_col")
            nc.gpsimd.memset(ones_col[:], 1.0)
            onef_row = cp.tile([1, 128], dt.float32, tag="onef_row")
            nc.gpsimd.memset(onef_row[:], 1.0)
            oneb_row = cp.tile([1, 128], dt.bfloat16, tag="oneb_row")
            nc.gpsimd.memset(oneb_row[:], 1.0)
            epsb = cp.tile([1, 1], dt.float32, tag="epsb")
            nc.gpsimd.memset(epsb[:], EPS)

            g = cp.tile([1, 1], dt.float32, tag="g")
            nc.scalar.activation(g[:], lns[:], AF.Sigmoid, scale=0.1)
            omg = cp.tile([1, 1], dt.float32, tag="omg")
            nc.scalar.activation(omg[:], g[:], AF.Copy, bias=1.0, scale=-1.0)
            romg = cp.tile([1, 1], dt.float32, tag="romg")
            nc.vector.reciprocal(romg[:], omg[:])
            gainsq = cp.tile([1, HQ], dt.float32, tag="gainsq")
            nc.vector.tensor_mul(gainsq[:], qkg[:], qkg[:])
            # scalar pack: [negC(2), gsc(2), r=g/(1-g), omg, lns] -> 7
            pack = cp.tile([1, 7], dt.float32, tag="pack")
            nc.scalar.activation(pack[:, 0:HQ], gainsq[:], AF.Copy, scale=-float(np.sqrt(HD)))
            nc.scalar.activation(pack[:, 2:4], gainsq[:], AF.Copy, scale=float(1.0 / np.sqrt(HD)))
            nc.vector.tensor_scalar_add(pack[:, 2:4], pack[:, 2:4], 1e-30)
            nc.vector.tensor_mul(pack[:, 4:5], g[:], romg[:])
            nc.vector.tensor_copy(pack[:, 5:6], omg[:])
            # + 0*junk: keeps the HAM-warm fillers alive through DCE
            nc.vector.tensor_add(pack[:, 6:7], lns[:], junk[:])
            with tc.tile_pool(name="bc_ps", bufs=1, space="PSUM") as bcp:
                pk_ps = bcp.tile([128, 7], dt.float32, tag="pk_ps")
                nc.tensor.matmul(pk_ps[:], lhsT=onef_row[:], rhs=pack[:], start=True, stop=True)
                sc128 = cp.tile([128, 7], dt.float32, tag="sc128")
                nc.vector.tensor_copy(sc128[:], pk_ps[:])
            negC = sc128[:, 0:2]
            gsc = sc128[:, 2:4]
            r128 = sc128[:, 4:5]
            omg128 = sc128[:, 5:6]
            lns128 = sc128[:, 6:7]

            # scaled v_bias (1-g)*v_bias, transposed to [128,1] via PE
            vbs = cp.tile([1, HD], dt.float32, tag="vbs")
            nc.vector.tensor_scalar_mul(vbs[:], vb[:], omg[:, 0:1])
            vbsT = cp.tile([128, 1], dt.float32, tag="vbsT")
            with tc.tile_pool(name="bc2_ps", bufs=1, space="PSUM") as bcp2:
                vb_ps = bcp2.tile([128, 1], dt.float32, tag="vb_ps")
                nc.tensor.matmul(vb_ps[:], lhsT=vbs[:], rhs=onef_row[:, 0:1], start=True, stop=True)
                nc.vector.tensor_copy(vbsT[:], vb_ps[:])

            # ---- weights + inputs: all DMAs issued up front, in need order
            # (wq+x0 gate the first q unit; cos/sin gate its epilogue;
            #  wk/wv+s0 gate the first k unit) ----
            wq_sb = cp.tile([128, n_dt * HQ * HD], dt.bfloat16, tag="wq_sb")
            for k in range(n_dt):
                nc.sync.dma_start(
                    out=wq_sb[:, k * HQ * HD : (k + 1) * HQ * HD],
                    in_=wqT_d[128 * k : 128 * (k + 1), :],
                )

            xp = tc.alloc_tile_pool(name="xin", bufs=1)
            hw_ = 1024
            xt = {}  # (half, k) -> x tile
            st = {}  # (half, k) -> skip tile, later blended to x + r*skip
            for k in range(n_dt):
                xx = xp.tile([128, hw_], dt.bfloat16, name="xx", tag=f"x0_{k}")
                nc.sync.dma_start(out=xx[:], in_=xT_d[128 * k : 128 * (k + 1), 0:hw_])
                xt[(0, k)] = xx

            cosF = cp.tile([128, t], dt.bfloat16, tag="cosF")
            sinF = cp.tile([128, t], dt.bfloat16, tag="sinF")
            nc.sync.dma_start(out=cosF[:], in_=cosF_d[:])
            nc.sync.dma_start(out=sinF[:], in_=sinF_d[:])

            wk_sb = cp.tile([128, n_dt * HD], dt.bfloat16, tag="wk_sb")
            wv_sb = cp.tile([128, n_dt * HD], dt.bfloat16, tag="wv_sb")
            for k in range(n_dt):
                nc.sync.dma_start(out=wk_sb[:, k * HD : (k + 1) * HD], in_=wkT_d[128 * k : 128 * (k + 1), :])
            for k in range(n_dt):
                nc.sync.dma_start(out=wv_sb[:, k * HD : (k + 1) * HD], in_=wvT_d[128 * k : 128 * (k + 1), :])
            # fold (1-g) into the k/v weights
            nc.vector.tensor_scalar_mul(wk_sb[:], wk_sb[:], omg128[:, 0:1])
            nc.vector.tensor_scalar_mul(wv_sb[:], wv_sb[:], omg128[:, 0:1])

            for hf in range(n_half):
                for k in range(n_dt):
                    ss = xp.tile([128, hw_], dt.bfloat16, name="ss", tag=f"s{hf}_{k}")
                    nc.sync.dma_start(out=ss[:], in_=skT_d[128 * k : 128 * (k + 1), hw_ * hf : hw_ * (hf + 1)])
                    st[(hf, k)] = ss
                if hf == 0:
                    for k in range(n_dt):
                        xx = xp.tile([128, hw_], dt.bfloat16, name="xx", tag=f"x1_{k}")
                        nc.sync.dma_start(out=xx[:], in_=xT_d[128 * k : 128 * (k + 1), hw_ : 2 * hw_])
                        xt[(1, k)] = xx

            # ---- persistent activations ----
            kT = cp.tile([128, t], dt.bfloat16, tag="kT")
            vT_sb = cp.tile([128, t], dt.bfloat16, tag="vT_sb")
            vnat = cp.tile([128, t], dt.bfloat16, tag="vnat")
            qT = cp.tile([128, HQ * t], dt.bfloat16, tag="qT")

            # ---- phase 1: projections + norm/rope epilogues ----
            with tc.tile_pool(name="p1s", bufs=2) as sp:
                pend = deque()

                def drain(nmax):
                    while len(pend) > nmax:
                        pend.popleft()()

                def stage1_norm(ps, cs):
                    """sum of squares -> [1,512] row; sqrt of mean+eps."""
                    sq = sp.tile([128, 512], dt.bfloat16, tag="sq")
                    nc.scalar.square(sq[:], ps[:])
                    ssq = ssqp.tile([1, 512], dt.float32, tag="ssq")
                    nc.tensor.matmul(ssq[:], lhsT=ones_col[:], rhs=sq[:], start=True, stop=True)
                    srms = sp.tile([1, 512], dt.bfloat16, tag="srms")
                    nc.scalar.activation(srms[:], ssq[:], AF.Sqrt, bias=epsb[0:1, 0:1], scale=1.0 / HD)
                    return srms

                def stage2_norm(ps, cs, dest, srms):
                    rb = rbp.tile([128, 512], dt.float32, tag="rb")
                    nc.tensor.matmul(rb[:], lhsT=oneb_row[:], rhs=srms[:], start=True, stop=True)
                    rr = sp.tile([128, 512], dt.float32, tag="rr")
                    nc.vector.reciprocal_approx_fast(rr[:], rb[:])
                    qh = sp.tile([128, 512], dt.bfloat16, tag="qh")
                    nc.vector.tensor_mul(qh[:], ps[:], rr[:])
                    qsw = sp.tile([128, 512], dt.bfloat16, tag="qsw")
                    nc.vector.tensor_copy(qsw[0:64, :], qh[64:128, :])
                    nc.vector.tensor_copy(qsw[64:128, :], qh[0:64, :])
                    # tsw mul + final add on gpsimd (idle in phase 1) to
                    # unload the saturated DVE
                    tsw = sp.tile([128, 512], dt.bfloat16, tag="tsw")
                    nc.gpsimd.tensor_mul(tsw[:], qsw[:], sinF[:, cs])
                    tco = sp.tile([128, 512], dt.bfloat16, tag="tco")
                    nc.vector.tensor_mul(tco[:], qh[:], cosF[:, cs])
                    nc.gpsimd.tensor_add(dest, tco[:], tsw[:])

                for hf in range(n_half):
                    h0 = hw_ * hf
                    units = [("q", 0, 0), ("q", 0, 1), ("blend", None, None), ("q", 1, 0), ("q", 1, 1)]
                    units += [("k", None, cl) for cl in range(2)]
                    units += [("v", None, cl) for cl in range(2)]
                    for kind, h, cl in units:
                        if kind == "blend":
                            # in-place fused: skip <- (skip * r + 0) + x  (one
                            # DVE op per tile; placed here so it neither blocks
                            # early epilogues nor delays the k units)
                            for k in range(n_dt):
                                nc.vector.affine_then_add(
                                    out=st[(hf, k)][:], in0=st[(hf, k)][:], in1=xt[(hf, k)][:],
                                    scale=r128[:, 0:1], bias=0.0,
                                )
                            continue
                        cs = slice(h0 + 512 * cl, h0 + 512 * (cl + 1))
                        ls = slice(512 * cl, 512 * (cl + 1))
                        ps = psp.tile([128, 512], dt.float32, tag="proj_ps")
                        if kind == "q":
                            dest = qT[:, t * h + h0 + 512 * cl : t * h + h0 + 512 * (cl + 1)]
                            for k in range(n_dt):
                                nc.tensor.matmul(
                                    ps[:],
                                    lhsT=wq_sb[:, k * HQ * HD + h * HD : k * HQ * HD + (h + 1) * HD],
                                    rhs=xt[(hf, k)][:, ls],
                                    start=(k == 0),
                                    stop=(k == n_dt - 1),
                                )
                        else:
                            w_sb = wk_sb if kind == "k" else wv_sb
                            dest = (kT if kind == "k" else vT_sb)[:, cs]
                            for k in range(n_dt):
                                nc.tensor.matmul(
                                    ps[:],
                                    lhsT=w_sb[:, k * HD : (k + 1) * HD],
                                    rhs=st[(hf, k)][:, ls],
                                    start=(k == 0),
                                    stop=(k == n_dt - 1),
                                )
                        if kind == "v":
                            def fin_v(ps=ps, dest=dest):
                                # ACT copy: keeps the psum evacuation off the DVE
                                nc.scalar.activation(dest, ps[:], AF.Copy)
                            pend.append(fin_v)
                        else:
                            state = {}
                            def s1(ps=ps, cs=cs, state=state):
                                state["srms"] = stage1_norm(ps, cs)
                            def s2(ps=ps, cs=cs, dest=dest, state=state):
                                stage2_norm(ps, cs, dest, state["srms"])
                            pend.append(s1)
                            pend.append(s2)
                        drain(3)
                    drain(0)
                    # v natural layout: add bias then XBAR dma-transpose
                    hs = slice(h0, h0 + hw_)
                    nc.vector.tensor_scalar_add(vT_sb[:, hs], vT_sb[:, hs], vbsT[:, 0:1])
                    for kt in range(8 * hf, 8 * (hf + 1)):
                        nc.sync.dma_start_transpose(
                            out=vnat[:, HD * kt : HD * (kt + 1)],
                            in_=vT_sb[:, 128 * kt : 128 * (kt + 1)],
                        )
            xp.release()
            rbp.release()
            ssqp.release()
            psp.release()

            # wproj stream on the sync (hardware-DGE) queue: one strided DMA
            # per 2MB block so the gpsimd queue stays clear for the A2As
            prp = tc.alloc_tile_pool(name="pr_s", bufs=1)
            wps = []
            for n in range(D // 512):
                wp = prp.tile([128, n_dt * 512], dt.bfloat16, name=f"wp{n}", tag=f"wp{n}")
                for a in range(n_dt):
                    nc.sync.dma_start(
                        out=wp[:, 512 * a : 512 * (a + 1)],
                        in_=wpT_d[128 * a : 128 * (a + 1), 512 * n : 512 * (n + 1)],
                    )
                wps.append(wp)

            # ---- phase 2: attention ----
            y_in = [dp.tile([NCORES, HD, rows], dt.bfloat16, name=f"y_in{h}", tag=f"y_in{h}") for h in range(HQ)]
            y_out = [dp.tile([NCORES, HD, rows], dt.bfloat16, name=f"y_out{h}", tag=f"y_out{h}") for h in range(HQ)]
            ytp_ = tc.alloc_tile_pool(name="yt_s", bufs=1)
            yt_blocks = {}
            with (
                tc.tile_pool(name="att_s", bufs=4) as ap_,
                tc.tile_pool(name="att_s2", bufs=2) as ap2,
                tc.tile_pool(name="st_ps", bufs=2, space="PSUM") as stp_,
                tc.tile_pool(name="yl_ps", bufs=2, space="PSUM") as ylp_,
                tc.tile_pool(name="l_ps", bufs=1, space="PSUM") as lp_,
            ):
                pend2 = deque()  # entries: (is_post, closure)

                def drain2(nmax):
                    # depth counts only 'post' (exp-consumer) entries so a
                    # cheap chunk-end entry doesn't collapse the exp pipeline
                    while sum(1 for k, _ in pend2 if k) > nmax:
                        pend2.popleft()[1]()
                    while pend2 and not pend2[0][0] and nmax == 0:
                        pend2.popleft()[1]()

                for h in range(HQ):
                    for c in range(n_chunk):
                        qs = slice(t * h + 512 * c, t * h + 512 * (c + 1))
                        nkts = kpc * (c + 1)
                        ytp = ylp_.tile([128, 512], dt.float32, tag="yt")
                        lrow = lp_.tile([1, 512], dt.float32, tag="l")
                        for pg in range((nkts + 1) // 2):
                            kts = [kk for kk in (2 * pg, 2 * pg + 1) if kk < nkts]
                            w = 512 * len(kts)
                            stp = stp_.tile([128, 1024], dt.float32, tag="st")
                            for s, kt in enumerate(kts):
                                nc.tensor.matmul(
                                    stp[:, 512 * s : 512 * s + 512],
                                    lhsT=kT[:, 128 * kt : 128 * (kt + 1)],
                                    rhs=qT[:, qs],
                                    start=True,
                                    stop=True,
                                )
                            pp = ap_.tile([128, 1024], dt.bfloat16, tag="pp")
                            nc.scalar.activation(
                                pp[:, :w], stp[:, :w], AF.Exp,
                                bias=negC[:, h : h + 1], scale=gsc[:, h : h + 1],
                            )

                            def post(pp=pp, kts=kts, c=c, ytp=ytp, lrow=lrow, nkts=nkts):
                                for s, kt in enumerate(kts):
                                    seg = pp[:, 512 * s : 512 * s + 512]
                                    if kt >= kpc * c:
                                        m = kt - kpc * c
                                        nc.vector.tensor_mul(seg, seg, mask[:, 512 * m : 512 * (m + 1)])
                                for s, kt in enumerate(kts):
                                    seg = pp[:, 512 * s : 512 * s + 512]
                                    nc.tensor.matmul(
                                        lrow[:], lhsT=ones_col[:], rhs=seg,
                                        start=(kt == 0), stop=(kt == nkts - 1), skip_group_check=True,
                                    )
                                    nc.tensor.matmul(
                                        ytp[:], lhsT=vnat[:, HD * kt : HD * (kt + 1)], rhs=seg,
                                        start=(kt == 0), stop=(kt == nkts - 1),
                                    )
                            pend2.append((True, post))
                            drain2(2)

                        def chunk_end(ytp=ytp, lrow=lrow, h=h, c=c):
                            lsb = ap2.tile([1, 512], dt.bfloat16, tag="lsb")
                            nc.vector.tensor_copy(lsb[:], lrow[:])
                            rbps = stp_.tile([128, 1024], dt.float32, tag="st")
                            nc.tensor.matmul(rbps[:, 0:512], lhsT=oneb_row[:], rhs=lsb[:], start=True, stop=True)
                            rl = ap2.tile([128, 512], dt.float32, tag="rl")
                            nc.vector.reciprocal_approx_fast(rl[:], rbps[:, 0:512])
                            ysb = ap2.tile([128, 512], dt.bfloat16, tag="ysb")
                            nc.vector.tensor_mul(ysb[:], ytp[:], rl[:])
                            for b in range(512 // rows):
                                piece = (512 * c) // rows + b
                                nc.sync.dma_start(
                                    out=y_in[h][piece, :, :],
                                    in_=ysb[:, rows * b : rows * (b + 1)],
                                )
                        pend2.append((False, chunk_end))
                    drain2(0)
                    nc.gpsimd.collective_compute(
                        "AllToAll",
                        ALU.bypass,
                        replica_groups=[list(range(NCORES))],
                        ins=[y_in[h].opt()],
                        outs=[y_out[h].opt()],
                    )
                    # gather this head's din-blocks (gpsimd queue, after A2A)
                    for a in range(h, n_dt, HQ):
                        j = a // HQ
                        yb = ytp_.tile([128, rows], dt.bfloat16, name=f"ytb{a}", tag=f"ytb{a}")
                        nc.gpsimd.dma_start(out=yb[:], in_=y_out[h][j, :, :])
                        yt_blocks[a] = yb

            # ---- phase 3: output projection (head-0 blocks first) ----
            with (
                tc.tile_pool(name="pr_ps", bufs=1, space="PSUM") as prps,
                tc.tile_pool(name="pr_o", bufs=2) as prout,
            ):
                groups = [(n, b) for n in range(D // 512) for b in range(rows // 128)]
                opses = []
                for gi, (n, b) in enumerate(groups):
                    ops = prps.tile([128, 512], dt.float32, tag=f"ops{gi}")
                    for ai, a in enumerate(range(0, n_dt, HQ)):
                        nc.tensor.matmul(
                            ops[:],
                            lhsT=yt_blocks[a][:, 128 * b : 128 * (b + 1)],
                            rhs=wps[n][:, 512 * a : 512 * (a + 1)],
                            start=(ai == 0),
                            stop=False,
                            skip_group_check=True,
                        )
                    opses.append(ops)
                for gi, (n, b) in enumerate(groups):
                    nblk = list(range(1, n_dt, HQ))
                    for ai, a in enumerate(nblk):
                        nc.tensor.matmul(
                            opses[gi][:],
                            lhsT=yt_blocks[a][:, 128 * b : 128 * (b + 1)],
                            rhs=wps[n][:, 512 * a : 512 * (a + 1)],
                            start=False,
                            stop=(ai == len(nblk) - 1),
                            skip_group_check=True,
                        )
                    osb = prout.tile([128, 512], dt.float32, tag="osb")
                    nc.vector.tensor_scalar_mul(osb[:], opses[gi][:], lns128[:, 0:1])
                    nc.sync.dma_start(
                        out=out_d[128 * b : 128 * (b + 1), 512 * n : 512 * (n + 1)],
                        in_=osb[:],
                    )
            ytp_.release()
            prp.release()
    nc.finalize()
    return nc


def make_tables(t=T):
    pos = np.arange(t, dtype=np.float32)
    inv = 1.0 / (ROPE_BASE ** (np.arange(0, HD, 2, dtype=np.float32) / HD))
    fr = pos[:, None] * inv[None, :]  # [t, 64]
    cos = np.cos(fr).T  # [64, t]
    sin = np.sin(fr).T
    cosF = np.concatenate([cos, cos], axis=0)  # [128, t]
    sinF = np.concatenate([sin, -sin], axis=0)
    return _bf(cosF), _bf(sinF)


def make_masks():
    # binary mask: maskb[p, 512*m + j] = 1 if j >= 128*m + p else 0
    p = np.arange(128)[:, None]
    j = np.arange(512)[None, :]
    blocks = [np.where(j >= 128 * m + p, 1.0, 0.0) for m in range(4)]
    return _bf(np.concatenate(blocks, axis=1))


_GRAPH_CACHE = {}
_LAST_IN_MAPS = None


def kernel(x, skip, wq, wk, wv, wproj, qk_g, ln_s, v_bias):
    t = x.shape[1]
    if t not in _GRAPH_CACHE:
        _GRAPH_CACHE[t] = build_graph(t)
    nc = _GRAPH_CACHE[t]

    xT = _bf(x.reshape(t, D).T)
    skT = _bf(skip.reshape(t, D).T)
    wpT = _bf(np.asarray(wproj, np.float32).T)
    cosF, sinF = make_tables(t)
    maskb = make_masks()
    ident = _bf(np.eye(128, dtype=np.float32))

    in_maps = []
    for c in range(NCORES):
        kv = c // 2
        in_maps.append(
            {
                "xT": xT,
                "skipT": skT,
                "wqT": _bf(np.asarray(wq, np.float32)[HQ * HD * c : HQ * HD * (c + 1), :].T),
                "wkT": _bf(np.asarray(wk, np.float32)[HD * kv : HD * (kv + 1), :].T),
                "wvT": _bf(np.asarray(wv, np.float32)[HD * kv : HD * (kv + 1), :].T),
                "wprojT": wpT,
                "qkg": np.asarray(qk_g, np.float32)[HQ * c : HQ * (c + 1)].reshape(1, HQ),
                "lns": np.asarray(ln_s, np.float32).reshape(1, 1),
                "vbias": np.asarray(v_bias, np.float32)[kv].reshape(1, HD),
                "cosF": cosF,
                "sinF": sinF,
                "maskb": maskb,
                "ident": ident,
            }
        )
    global _LAST_IN_MAPS
    _LAST_IN_MAPS = in_maps
    res = run_bass_kernel_spmd(nc, in_maps, list(range(NCORES)))
    out = np.concatenate(
        [np.asarray(res.results[c]["out"], np.float32) for c in range(NCORES)], axis=0
    )
    return out.reshape(1, t, D).astype(np.float32)
